# revision 1
# baseline (speedup 1.0000x reference)
"""BinaryDense Trainium2 kernel: out = nmk * (inputs @ binarize(weight).T + bias).

binarize(w) = tanh(w * kk) when kk < 1e6 else sign(w).

Strategy (column-parallel over 8 NeuronCores, per the tensor-parallel hint):
  - Each core owns a 2048-row slice of weight/bias (out_channels).
  - On device, the weight slice is streamed once (fp32), binarized with the
    scalar engine, and kept resident in SBUF as fp16 in 4 panels of 512 oc.
  - Inputs are transposed/cast to fp16 on the host (layout prep only) and
    streamed in 512-token chunks, once per panel (4x total).
  - Matmuls: stationary fp16 weight tile [k=128, oc=128], moving fp16 input
    tile [k=128, tok=512], fp32 PSUM accumulation over 32 k-tiles.
  - PSUM eviction fuses nmk*(acc + bias) in one DVE tensor_scalar op.
  - Per-core output is [oc, tok]; the host concatenates/transposes.
"""

import numpy as np

import concourse.bass as bass
import concourse.mybir as mybir
import concourse.tile as tile
from concourse.bass_utils import run_bass_kernel_spmd
from concourse.mybir import ActivationFunctionType, AluOpType

N_CORES = 8
P = 128
IN_CH = 4096
OUT_CH = 16384
TOKENS = 8192
KK_THRESHOLD = 1e6

KT = IN_CH // P          # 32 k-tiles of 128
OC_SH = OUT_CH // N_CORES  # 2048 out-channels per core
CHUNK = 512              # tokens per streamed input chunk
NCH = TOKENS // CHUNK    # 16 chunks
PANEL = 512              # out-channels per resident fp16 weight panel
NQ = OC_SH // PANEL      # 4 panels
OPT = PANEL // P         # 4 oc-tiles per panel
NOCT = OC_SH // P        # 16 oc-tiles per core


def _split_multi_waits(nc, cap=1):
    """Split instructions carrying more than `cap` sync waits.

    The walrus build in this environment supports a single sync-wait command
    per TPB instruction, but Tile's kernel-tail drain/barrier can accumulate
    several residual waits. Moving the excess onto preceding NoOps on the
    same engine is equivalent: the sequencer blocks on each wait in order.
    """
    for f in nc.m.functions:
        for bb in f.blocks:
            out = []
            for inst in bb.instructions:
                si = inst.sync_info
                waits = list(si.on_wait) if si is not None and si.on_wait else []
                if len(waits) > cap:
                    spill, keep = waits[:-cap], waits[-cap:]
                    for i in range(0, len(spill), cap):
                        noop = mybir.InstNoOp(
                            name=nc.get_next_instruction_name(),
                            ins=[],
                            outs=[],
                            engine=inst.engine,
                        )
                        noop.sync_info = mybir.SyncInfo(
                            on_wait=spill[i : i + cap], on_update=[]
                        )
                        nc.register_instruction(noop)
                        out.append(noop)
                    inst.sync_info = mybir.SyncInfo(
                        on_wait=keep,
                        on_update=list(si.on_update) if si.on_update else [],
                    )
                out.append(inst)
            bb.instructions = out


def _build(tanh_branch: bool):
    f32, f16 = mybir.dt.float32, mybir.dt.float16
    nc = bass.Bass("TRN2", target_bir_lowering=False, debug=False)
    # w6[q, ot, p, t*128+j] = weightT[t*128+p, q*PANEL + ot*128 + j]:
    # one oc-tile's whole K panel is contiguous per partition -> one DMA.
    # Stored fp16 (host layout/precision prep); tanh still runs on device.
    w6 = nc.dram_tensor(
        "w6", [NQ, OPT, P, KT * P], f16, kind="ExternalInput"
    ).ap()
    x4 = nc.dram_tensor("x4", [NCH, P, KT, CHUNK], f16, kind="ExternalInput").ap()
    bias_pt = nc.dram_tensor("bias_pt", [P, NOCT], f32, kind="ExternalInput").ap()
    nmk = nc.dram_tensor("nmk", [1], f32, kind="ExternalInput").ap()
    kk = nc.dram_tensor("kk", [1], f32, kind="ExternalInput").ap()
    o4 = nc.dram_tensor("o4", [NOCT, P, TOKENS], f32, kind="ExternalOutput").ap()

    with tile.TileContext(nc) as tc:
        with (
            tc.tile_pool(name="const", bufs=1) as constp,
            tc.tile_pool(name="wq", bufs=2 * OPT) as wqp,
            tc.tile_pool(name="xc", bufs=3) as xcp,
            tc.tile_pool(name="stage", bufs=4) as stp,
            tc.tile_pool(name="psum", bufs=8, space="PSUM") as psp,
        ):
            kk_b = constp.tile([P, 1], f32)
            nmk_b = constp.tile([P, 1], f32)
            nc.gpsimd.dma_start(out=kk_b[:], in_=kk.to_broadcast((P, 1)))
            nc.gpsimd.dma_start(out=nmk_b[:], in_=nmk.to_broadcast((P, 1)))
            bias_sb = constp.tile([P, NOCT], f32)
            nc.gpsimd.dma_start(out=bias_sb[:], in_=bias_pt[:])
            nb = constp.tile([P, NOCT], f32)  # nmk * bias, per oc-tile column
            nc.vector.tensor_scalar_mul(nb[:], bias_sb[:], nmk_b[:])

            # Prefetch the first input chunk: it now gates the first matmul
            # (the fp16 weight sub-panels are smaller), so it must not sit
            # behind them on the sync queue.
            xc_pre = xcp.tile([P, KT, CHUNK], f16, tag="xc")
            nc.sync.dma_start(out=xc_pre[:], in_=x4[0])

            for q in range(NQ):
                # One fp16 sub-panel tile per oc-tile: a single contiguous
                # DMA + a single big tanh each, so the first matmul group
                # only waits for the first 4.2MB sub-panel.
                wq = []
                for ot in range(OPT):
                    wsub = wqp.tile([P, KT * P], f16, tag="wsub")
                    # Split the 2.1MB load across two engine queue sets so
                    # the first panel lands in ~half the time; binarize
                    # in place (fp16 -> fp16).
                    if q == 0 and ot == 0:
                        # The first sub-panel gates the first matmul and must
                        # stay off the sync queue (owned by the input
                        # prefetch): split scalar + otherwise-idle gpsimd,
                        # sized for their measured bandwidths.
                        cut = KT * P * 5 // 8
                        nc.scalar.dma_start(
                            out=wsub[:, :cut], in_=w6[q, ot, :, :cut]
                        )
                        nc.gpsimd.dma_start(
                            out=wsub[:, cut:], in_=w6[q, ot, :, cut:]
                        )
                    else:
                        half = KT * P // 2
                        nc.scalar.dma_start(
                            out=wsub[:, :half], in_=w6[q, ot, :, :half]
                        )
                        nc.sync.dma_start(
                            out=wsub[:, half:], in_=w6[q, ot, :, half:]
                        )
                    if tanh_branch:
                        nc.scalar.activation(
                            wsub[:],
                            wsub[:],
                            ActivationFunctionType.Tanh,
                            scale=kk_b[:],
                        )
                    else:
                        nc.scalar.activation(
                            wsub[:], wsub[:], ActivationFunctionType.Sign
                        )
                    wq.append(wsub)
                for ch in range(NCH):
                    if q == 0 and ch == 0:
                        xc = xc_pre
                    else:
                        xc = xcp.tile([P, KT, CHUNK], f16, tag="xc")
                        nc.sync.dma_start(out=xc[:], in_=x4[ch])
                    for ot in range(OPT):
                        ps = psp.tile([P, CHUNK], f32)
                        for t in range(KT):
                            nc.tensor.matmul(
                                ps[:],
                                wq[ot][:, t * P : (t + 1) * P],
                                xc[:, t, :],
                                start=(t == 0),
                                stop=(t == KT - 1),
                            )
                        og = q * OPT + ot
                        st = stp.tile([P, CHUNK], f32)
                        nc.vector.tensor_scalar(
                            st[:],
                            ps[:],
                            nmk_b[:],
                            nb[:, og : og + 1],
                            op0=AluOpType.mult,
                            op1=AluOpType.add,
                        )
                        # Final chunk's stores ride the scalar HWDGE queue
                        # (idle by then, faster submit) to shorten the tail.
                        last = q == NQ - 1 and ch == NCH - 1
                        store_eng = nc.scalar if last else nc.gpsimd
                        store_eng.dma_start(
                            out=o4[og, :, ch * CHUNK : (ch + 1) * CHUNK], in_=st[:]
                        )

    _split_multi_waits(nc)
    return nc


_PROGRAM_CACHE = {}


def _get_program(tanh_branch: bool):
    if tanh_branch not in _PROGRAM_CACHE:
        _PROGRAM_CACHE[tanh_branch] = _build(tanh_branch)
    return _PROGRAM_CACHE[tanh_branch]


def _prep_inputs(inputs, weight, bias, nmk, kk):
    x = np.asarray(inputs, dtype=np.float32)
    w = np.asarray(weight, dtype=np.float32)
    b = np.asarray(bias, dtype=np.float32)
    nmk = np.asarray(nmk, dtype=np.float32).reshape(1)
    kk = np.asarray(kk, dtype=np.float32).reshape(1)

    # x4[c, p, t, j] = x[c*CHUNK + j, t*P + p], fp16
    xt = np.ascontiguousarray(x.T).astype(np.float16)  # [IN_CH, TOKENS]
    x4 = np.ascontiguousarray(
        xt.reshape(KT, P, NCH, CHUNK).transpose(2, 1, 0, 3)
    )

    in_maps = []
    for c in range(N_CORES):
        wsh = w[c * OC_SH : (c + 1) * OC_SH, :]  # [OC_SH, IN_CH]
        # w6[q, ot, p, t*P+j] = wsh.T[t*P+p, q*PANEL + ot*P + j]
        w6 = np.ascontiguousarray(
            np.ascontiguousarray(wsh.T)
            .reshape(KT, P, NQ, OPT, P)
            .transpose(2, 3, 1, 0, 4)
            .reshape(NQ, OPT, P, KT * P)
        ).astype(np.float16)
        bsh = np.ascontiguousarray(
            b[c * OC_SH : (c + 1) * OC_SH].reshape(NOCT, P).T
        )
        in_maps.append(
            {"w6": w6, "x4": x4, "bias_pt": bsh, "nmk": nmk, "kk": kk}
        )
    return in_maps, kk


def _run(inputs, weight, bias, nmk, kk, trace=False, tmpdir=None):
    in_maps, kk_arr = _prep_inputs(inputs, weight, bias, nmk, kk)
    nc = _get_program(bool(kk_arr[0] < KK_THRESHOLD))
    res = run_bass_kernel_spmd(
        nc, in_maps, core_ids=list(range(N_CORES)), trace=trace, tmpdir=tmpdir
    )
    out = np.empty((TOKENS, OUT_CH), dtype=np.float32)
    for c in range(N_CORES):
        o4 = res.results[c]["o4"]  # [NOCT, P, TOKENS]
        out[:, c * OC_SH : (c + 1) * OC_SH] = o4.reshape(OC_SH, TOKENS).T
    return out, res


def kernel(inputs, weight, bias, nmk, kk):
    out, _ = _run(inputs, weight, bias, nmk, kk, trace=False)
    return out



# revision 5
# speedup vs baseline: 1.1016x; 1.1016x over previous
"""BinaryDense Trainium2 kernel: out = nmk * (inputs @ binarize(weight).T + bias).

binarize(w) = tanh(w * kk) when kk < 1e6 else sign(w).

Strategy (column-parallel over 8 NeuronCores, per the tensor-parallel hint):
  - Each core owns a 2048-row slice of weight/bias (out_channels).
  - Hybrid-precision contraction: of the 32 k-tiles (128 each), the first 26
    run as fp16 matmuls (1 col/cycle) and the last 6 as 3 fp8e4m3 DoubleRow
    matmuls (2 k-tiles per pass at 2 cols/cycle), cutting tensor-engine time
    to 27.5/32 of the fp16 roofline. Measured end-to-end rel err 1.64e-2
    vs the 2e-2 budget (fp16-only is 3.7e-4; fp8-only would be 3.8e-2).
  - All matmuls accumulate into one PSUM bank: operands are pre-scaled so
    every product carries the same factor 512 (x16 = x*512 in fp16;
    x8 = fp8(x*32), w8 = fp8(binarize(w)*16)), and the eviction multiplies
    by nmk/512. fp16-part weights are binarized on device (scalar engine);
    the fp8 panel is quantized host-side (quantization must see the
    binarized values to place the mantissa).
  - Inputs are transposed/cast on the host (layout prep) and streamed in
    512-token chunks, once per 512-oc panel (4x total).
  - PSUM eviction fuses (nmk/512)*acc + nmk*bias in one DVE tensor_scalar.
  - Per-core output is [oc, tok]; the host concatenates/transposes.
"""

import ml_dtypes
import numpy as np

import concourse.bass as bass
import concourse.mybir as mybir
import concourse.tile as tile
from concourse.bass_utils import run_bass_kernel_spmd
from concourse.mybir import ActivationFunctionType, AluOpType

N_CORES = 8
P = 128
IN_CH = 4096
OUT_CH = 16384
TOKENS = 8192
KK_THRESHOLD = 1e6

OC_SH = OUT_CH // N_CORES  # 2048 out-channels per core
CHUNK = 512              # tokens per streamed input chunk
NCH = TOKENS // CHUNK    # 16 chunks
PANEL = 512              # out-channels per resident weight panel
NQ = OC_SH // PANEL      # 4 panels
OPT = PANEL // P         # 4 oc-tiles per panel
NOCT = OC_SH // P        # 16 oc-tiles per core

NP8 = 3                  # fp8 DoubleRow passes (2 k-tiles each)
KT16 = IN_CH // P - 2 * NP8  # 26 fp16 k-tiles
KCUT = KT16 * P          # k index where the fp8 range starts
X16_SCALE = 512.0        # fp16 x pre-scale (== X8_SCALE * W8_SCALE)
W8_SCALE = 20.75         # fp8 weight scale (scan minimum for uniform tanh(w))
X8_SCALE = X16_SCALE / W8_SCALE  # ~24.7: |x|*24.7 < 240 for |x| < 9.7 sigma
FP8_MAX = 240.0          # TRN float8e4 (ml_dtypes.float8_e4m3) saturation


def _split_multi_waits(nc, cap=1):
    """Split instructions carrying more than `cap` sync waits.

    The walrus build in this environment supports a single sync-wait command
    per TPB instruction, but Tile's kernel-tail drain/barrier can accumulate
    several residual waits. Moving the excess onto preceding NoOps on the
    same engine is equivalent: the sequencer blocks on each wait in order.
    """
    for f in nc.m.functions:
        for bb in f.blocks:
            out = []
            for inst in bb.instructions:
                si = inst.sync_info
                waits = list(si.on_wait) if si is not None and si.on_wait else []
                if len(waits) > cap:
                    spill, keep = waits[:-cap], waits[-cap:]
                    for i in range(0, len(spill), cap):
                        noop = mybir.InstNoOp(
                            name=nc.get_next_instruction_name(),
                            ins=[],
                            outs=[],
                            engine=inst.engine,
                        )
                        noop.sync_info = mybir.SyncInfo(
                            on_wait=spill[i : i + cap], on_update=[]
                        )
                        nc.register_instruction(noop)
                        out.append(noop)
                    inst.sync_info = mybir.SyncInfo(
                        on_wait=keep,
                        on_update=list(si.on_update) if si.on_update else [],
                    )
                out.append(inst)
            bb.instructions = out


def _build(tanh_branch: bool):
    f32, f16 = mybir.dt.float32, mybir.dt.float16
    f8 = mybir.dt.float8e4
    nc = bass.Bass("TRN2", target_bir_lowering=False, debug=False)
    # w16[q, ot, p, t*128+j] = weightT[t*128+p, q*PANEL + ot*128 + j]:
    # one oc-tile's whole fp16 K panel is contiguous per partition -> one DMA.
    # Stored fp16 (host layout/precision prep); tanh still runs on device.
    w16d = nc.dram_tensor(
        "w16", [NQ, OPT, P, KT16 * P], f16, kind="ExternalInput"
    ).ap()
    # w8[q, ot, p, j, i, m] = fp8(binarize(weightT[(KT16+2j+i)*128+p,
    #                                              q*PANEL+ot*128+m]) * 16)
    w8d = nc.dram_tensor(
        "w8", [NQ, OPT, P, NP8, 2, P], f8, kind="ExternalInput"
    ).ap()
    x16d = nc.dram_tensor(
        "x16", [NCH, P, KT16, CHUNK], f16, kind="ExternalInput"
    ).ap()
    x8d = nc.dram_tensor(
        "x8", [NCH, P, NP8, 2, CHUNK], f8, kind="ExternalInput"
    ).ap()
    bias_pt = nc.dram_tensor("bias_pt", [P, NOCT], f32, kind="ExternalInput").ap()
    nmk = nc.dram_tensor("nmk", [1], f32, kind="ExternalInput").ap()
    nmk_s = nc.dram_tensor("nmk_s", [1], f32, kind="ExternalInput").ap()
    kk = nc.dram_tensor("kk", [1], f32, kind="ExternalInput").ap()
    o4 = nc.dram_tensor("o4", [NOCT, P, TOKENS], f32, kind="ExternalOutput").ap()

    with tile.TileContext(nc) as tc:
        with (
            tc.tile_pool(name="const", bufs=1) as constp,
            tc.tile_pool(name="wq", bufs=2 * OPT) as wqp,
            tc.tile_pool(name="w8q", bufs=2 * OPT) as w8qp,
            tc.tile_pool(name="xc", bufs=3) as xcp,
            tc.tile_pool(name="x8c", bufs=3) as x8cp,
            tc.tile_pool(name="stage", bufs=4) as stp,
            tc.tile_pool(name="psum", bufs=8, space="PSUM") as psp,
        ):
            kk_b = constp.tile([P, 1], f32)
            nmk_b = constp.tile([P, 1], f32)
            nmk_s_b = constp.tile([P, 1], f32)
            nc.gpsimd.dma_start(out=kk_b[:], in_=kk.to_broadcast((P, 1)))
            nc.gpsimd.dma_start(out=nmk_b[:], in_=nmk.to_broadcast((P, 1)))
            nc.gpsimd.dma_start(out=nmk_s_b[:], in_=nmk_s.to_broadcast((P, 1)))
            bias_sb = constp.tile([P, NOCT], f32)
            nc.gpsimd.dma_start(out=bias_sb[:], in_=bias_pt[:])
            nb = constp.tile([P, NOCT], f32)  # nmk * bias, per oc-tile column
            nc.vector.tensor_scalar_mul(nb[:], bias_sb[:], nmk_b[:])

            # Prefetch the first input chunk: it gates the first matmul
            # group, so it must not sit behind the weight panels on the
            # sync queue.
            xc_pre = xcp.tile([P, KT16, CHUNK], f16, tag="xc")
            nc.sync.dma_start(out=xc_pre[:], in_=x16d[0])
            x8c_pre = x8cp.tile([P, NP8, 2, CHUNK], f8, tag="x8c")
            nc.sync.dma_start(out=x8c_pre[:], in_=x8d[0])

            for q in range(NQ):
                # One fp16 sub-panel tile per oc-tile: a single contiguous
                # DMA + a single big tanh each, so the first matmul group
                # only waits for the first sub-panel.
                wq16 = []
                wq8 = []
                for ot in range(OPT):
                    wsub = wqp.tile([P, KT16 * P], f16, tag="wsub")
                    # Split the load across two engine queue sets so the
                    # first panel lands in ~half the time; binarize in
                    # place (fp16 -> fp16).
                    if q == 0 and ot == 0:
                        # The first sub-panel gates the first matmul and must
                        # stay off the sync queue (owned by the input
                        # prefetch): split scalar + otherwise-idle gpsimd,
                        # sized for their measured bandwidths.
                        cut = KT16 * P * 5 // 8
                        nc.scalar.dma_start(
                            out=wsub[:, :cut], in_=w16d[q, ot, :, :cut]
                        )
                        nc.gpsimd.dma_start(
                            out=wsub[:, cut:], in_=w16d[q, ot, :, cut:]
                        )
                    else:
                        half = KT16 * P // 2
                        nc.scalar.dma_start(
                            out=wsub[:, :half], in_=w16d[q, ot, :, :half]
                        )
                        nc.sync.dma_start(
                            out=wsub[:, half:], in_=w16d[q, ot, :, half:]
                        )
                    if tanh_branch:
                        nc.scalar.activation(
                            wsub[:],
                            wsub[:],
                            ActivationFunctionType.Tanh,
                            scale=kk_b[:],
                        )
                    else:
                        nc.scalar.activation(
                            wsub[:], wsub[:], ActivationFunctionType.Sign
                        )
                    wq16.append(wsub)
                    # fp8 sub-panel: host-binarized and quantized; tiny
                    # (768B/partition), rides the gpsimd queue.
                    w8sub = w8qp.tile([P, NP8, 2, P], f8, tag="w8sub")
                    nc.gpsimd.dma_start(out=w8sub[:], in_=w8d[q, ot])
                    wq8.append(w8sub)
                for ch in range(NCH):
                    if q == 0 and ch == 0:
                        xc = xc_pre
                        x8c = x8c_pre
                    else:
                        xc = xcp.tile([P, KT16, CHUNK], f16, tag="xc")
                        nc.sync.dma_start(out=xc[:], in_=x16d[ch])
                        x8c = x8cp.tile([P, NP8, 2, CHUNK], f8, tag="x8c")
                        nc.sync.dma_start(out=x8c[:], in_=x8d[ch])
                    for ot in range(OPT):
                        ps = psp.tile([P, CHUNK], f32)
                        for t in range(KT16):
                            nc.tensor.matmul(
                                ps[:],
                                wq16[ot][:, t * P : (t + 1) * P],
                                xc[:, t, :],
                                start=(t == 0),
                                stop=False,
                            )
                        for j in range(NP8):
                            nc.tensor.matmul(
                                ps[:],
                                wq8[ot][:, j, :, :],
                                x8c[:, j, :, :],
                                start=False,
                                stop=(j == NP8 - 1),
                                perf_mode=mybir.MatmulPerfMode.DoubleRow,
                            )
                        og = q * OPT + ot
                        st = stp.tile([P, CHUNK], f32)
                        nc.vector.tensor_scalar(
                            st[:],
                            ps[:],
                            nmk_s_b[:],
                            nb[:, og : og + 1],
                            op0=AluOpType.mult,
                            op1=AluOpType.add,
                        )
                        # Final chunk's stores ride the scalar HWDGE queue
                        # (idle by then, faster submit) to shorten the tail.
                        last = q == NQ - 1 and ch == NCH - 1
                        store_eng = nc.scalar if last else nc.gpsimd
                        store_eng.dma_start(
                            out=o4[og, :, ch * CHUNK : (ch + 1) * CHUNK], in_=st[:]
                        )

    _split_multi_waits(nc)
    return nc


_PROGRAM_CACHE = {}


def _get_program(tanh_branch: bool):
    if tanh_branch not in _PROGRAM_CACHE:
        _PROGRAM_CACHE[tanh_branch] = _build(tanh_branch)
    return _PROGRAM_CACHE[tanh_branch]


def _q8(a: np.ndarray, scale: float) -> np.ndarray:
    return np.clip(a * scale, -FP8_MAX, FP8_MAX).astype(ml_dtypes.float8_e4m3)


def _prep_inputs(inputs, weight, bias, nmk, kk):
    x = np.asarray(inputs, dtype=np.float32)
    w = np.asarray(weight, dtype=np.float32)
    b = np.asarray(bias, dtype=np.float32)
    nmk = np.asarray(nmk, dtype=np.float32).reshape(1)
    kk = np.asarray(kk, dtype=np.float32).reshape(1)
    tanh_branch = bool(kk[0] < KK_THRESHOLD)
    nmk_s = (nmk / X16_SCALE).astype(np.float32)

    xt = np.ascontiguousarray(x.T)  # [IN_CH, TOKENS] f32
    # x16[c, p, t, j] = x[c*CHUNK + j, t*P + p] * 512, fp16
    x16 = np.ascontiguousarray(
        (xt[:KCUT] * X16_SCALE)
        .astype(np.float16)
        .reshape(KT16, P, NCH, CHUNK)
        .transpose(2, 1, 0, 3)
    )
    # x8[c, p, j, i, n] = fp8(x[c*CHUNK + n, (KT16 + 2j + i)*P + p] * 32)
    x8 = np.ascontiguousarray(
        _q8(xt[KCUT:], X8_SCALE)
        .reshape(NP8, 2, P, NCH, CHUNK)
        .transpose(3, 2, 0, 1, 4)
    )

    in_maps = []
    for c in range(N_CORES):
        wsh = w[c * OC_SH : (c + 1) * OC_SH, :]  # [OC_SH, IN_CH]
        wshT = np.ascontiguousarray(wsh.T)  # [IN_CH, OC_SH]
        # w16[q, ot, p, t*P+j] = wsh.T[t*P+p, q*PANEL + ot*P + j]
        w16 = np.ascontiguousarray(
            wshT[:KCUT]
            .astype(np.float16)
            .reshape(KT16, P, NQ, OPT, P)
            .transpose(2, 3, 1, 0, 4)
            .reshape(NQ, OPT, P, KT16 * P)
        )
        wtail = wshT[KCUT:]  # [2*NP8*P, OC_SH]
        wbin = np.tanh(wtail * kk[0]) if tanh_branch else np.sign(wtail)
        w8 = np.ascontiguousarray(
            _q8(wbin, W8_SCALE)
            .reshape(NP8, 2, P, NQ, OPT, P)
            .transpose(3, 4, 2, 0, 1, 5)
        )
        bsh = np.ascontiguousarray(
            b[c * OC_SH : (c + 1) * OC_SH].reshape(NOCT, P).T
        )
        in_maps.append(
            {
                "w16": w16,
                "w8": w8,
                "x16": x16,
                "x8": x8,
                "bias_pt": bsh,
                "nmk": nmk,
                "nmk_s": nmk_s,
                "kk": kk,
            }
        )
    return in_maps, kk


def _run(inputs, weight, bias, nmk, kk, trace=False, tmpdir=None):
    in_maps, kk_arr = _prep_inputs(inputs, weight, bias, nmk, kk)
    nc = _get_program(bool(kk_arr[0] < KK_THRESHOLD))
    res = run_bass_kernel_spmd(
        nc, in_maps, core_ids=list(range(N_CORES)), trace=trace, tmpdir=tmpdir
    )
    out = np.empty((TOKENS, OUT_CH), dtype=np.float32)
    for c in range(N_CORES):
        o4 = res.results[c]["o4"]  # [NOCT, P, TOKENS]
        out[:, c * OC_SH : (c + 1) * OC_SH] = o4.reshape(OC_SH, TOKENS).T
    return out, res


def kernel(inputs, weight, bias, nmk, kk):
    out, _ = _run(inputs, weight, bias, nmk, kk, trace=False)
    return out
